# revision 20
# baseline (speedup 1.0000x reference)
"""Causal dot-product attention (B=2, H=16, S=2048, D=64, fp32) on 8 NeuronCores.

Sharding: the 32 (batch, head) slices are split 4-per-core. Each head is
computed flash-attention style but transposed: scores are built as
S^T[k, q] = K_tile @ Q^T so that exp(S^T) lands in SBUF already in the
[k-partition, q-free] layout the PV matmul needs as its moving operand —
no on-chip transposes anywhere. The softmax denominator rides along as a
ones-column appended to V (output row 64), and the final divide + layout
transpose happen on the host.
"""

import numpy as np

B, H, S, D = 2, 16, 2048, 64
N_CORES = 8
HPC = (B * H) // N_CORES  # heads per core = 4
PAIRS = HPC // 2          # head pairs per core = 2
QB = 512                  # query block (free dim of the S^T matmul)
KT = 128                  # key tile (partition dim of S^T)
NQB = S // QB             # 4
NKT = S // KT             # 16
VC = D + 1                # V columns + ones column = 65
STRIP = 1024              # PSUM strip width (2 banks): one k-tile pair
SCALE = 1.0 / 8.0         # 1/sqrt(D)

_CACHE = {}


def _build():
    import concourse.mybir as mybir
    import concourse.tile as tile
    from concourse import bacc

    f32 = mybir.dt.float32
    f32r = mybir.dt.float32r
    nc = bacc.Bacc("TRN2")

    qt_d = nc.dram_tensor("qt", [PAIRS, 128, S], f32r, kind="ExternalInput")
    kt_d = nc.dram_tensor("kt", [PAIRS, 128, S], f32r, kind="ExternalInput")
    v_d = nc.dram_tensor("v", [PAIRS, 128, 2 * NKT * VC], f32r, kind="ExternalInput")
    out_d = nc.dram_tensor("out", [HPC, NQB, VC, QB], f32, kind="ExternalOutput")

    qt_ap = qt_d.ap()
    kt_ap = kt_d.ap()
    v_ap = v_d.ap()
    out_ap = out_d.ap()

    with tile.TileContext(nc) as tc:
        with (
            tc.tile_pool(name="const", bufs=1) as constp,
            tc.tile_pool(name="inp", bufs=1) as inp,
            tc.tile_pool(name="pt", bufs=6) as ptp,
            tc.tile_pool(name="ob", bufs=2) as obp,
            tc.tile_pool(name="st", bufs=3, space="PSUM") as stp,
            tc.tile_pool(name="ops", bufs=2, space="PSUM") as opsp,
        ):
            # U[p, j] = 1.0 if j >= p else 0.0 — causal mask for diagonal tiles
            u = constp.tile([128, QB], f32)
            nc.gpsimd.memset(u[:], 1.0)
            nc.gpsimd.affine_select(
                out=u[:],
                in_=u[:],
                compare_op=mybir.AluOpType.is_ge,
                fill=0.0,
                base=0,
                pattern=[[1, QB]],
                channel_multiplier=-1,
            )

            qt_sbs, kt_sbs, v_sbs = [], [], []
            for pair in range(PAIRS):
                qt_sb = inp.tile([128, S], f32r, tag=f"qt{pair}")
                kt_sb = inp.tile([128, S], f32r, tag=f"kt{pair}")
                v_sb = inp.tile([128, 2 * NKT * VC], f32r, tag=f"v{pair}")
                qt_sbs.append(qt_sb)
                kt_sbs.append(kt_sb)
                v_sbs.append(v_sb)
                # chunked loads so the first compute block starts early;
                # qb loop runs descending so Q chunks load high-to-low
                for sl, qsl in [
                    (slice(0, 256), slice(0, 512)),
                    (slice(256, 1024), slice(1536, 2048)),
                    (slice(1024, 1536), slice(1024, 1536)),
                    (slice(1536, 2048), slice(512, 1024)),
                ]:
                    nc.sync.dma_start(kt_sb[:, sl], kt_ap[pair, :, sl])
                    nc.sync.dma_start(qt_sb[:, qsl], qt_ap[pair, :, qsl])
                for h2 in range(2):
                    for i in range(4):
                        vsl = slice(
                            (h2 * NKT + i * 4) * VC, (h2 * NKT + (i + 1) * 4) * VC
                        )
                        nc.sync.dma_start(v_sb[:, vsl], v_ap[pair, :, vsl])

            for h in range(HPC):
                pair, h2 = divmod(h, 2)
                qt_sb = qt_sbs[pair]
                kt_sb = kt_sbs[pair]
                v_sb = v_sbs[pair]
                p0 = 64 * h2  # partition offset of this head's d-rows
                for qb in [0, 3, 2, 1]:
                    if True:
                        qs = qb * QB
                        nkt = 4 * qb + 4  # causal k-tiles for this q block
                        o_ps = opsp.tile([VC, QB], f32, tag="o")
                        # k-tiles stream through shared [128,1024] PSUM
                        # strips so exp runs as few wide ACT instructions as
                        # possible: full-tile pairs -> one [1024] exp; the 4
                        # diagonal (straddle) tiles -> two strips packed
                        # contiguously ([512+384] and [256+128]), scheduled
                        # first so their longer serial chain overlaps the
                        # full-tile pipeline.
                        groups = []  # (slots, exp spans) ; slot = (kt, off, w)
                        d = 4 * qb
                        groups.append(
                            ([(d, 0, QB), (d + 1, QB, QB - KT)], [(0, 2 * QB - KT)])
                        )
                        groups.append(
                            (
                                [(d + 2, 0, QB - 2 * KT), (d + 3, QB - 2 * KT, QB - 3 * KT)],
                                [(0, 2 * QB - 5 * KT)],
                            )
                        )
                        for g in range(2 * qb):
                            groups.append(
                                (
                                    [(2 * g, 0, QB), (2 * g + 1, QB, QB)],
                                    [(0, 2 * QB)],
                                )
                            )
                        order = [kt for slots, _ in groups for kt, _, _ in slots]
                        first_kt, last_kt = order[0], order[-1]
                        for slots, spans in groups:
                            st = stp.tile([128, STRIP], f32, tag="st")
                            pt = ptp.tile([128, STRIP], f32r, tag="pt")
                            for kt, off, w in slots:
                                nc.tensor.matmul(
                                    st[:, off : off + w],
                                    kt_sb[p0 : p0 + 64, kt * KT : kt * KT + KT],
                                    qt_sb[p0 : p0 + 64, qs + QB - w : qs + QB],
                                    start=True,
                                    stop=True,
                                )
                            for s0, s1 in spans:
                                nc.scalar.activation(
                                    pt[:, s0:s1],
                                    st[:, s0:s1],
                                    mybir.ActivationFunctionType.Exp,
                                    scale=SCALE,
                                )
                            for kt, off, w in slots:
                                if w < QB or kt == 4 * qb:  # diagonal tile
                                    # only cols [0:128) of the slot contain
                                    # the causal triangle (col >= 128 > any p)
                                    mw = min(w, KT)
                                    nc.vector.tensor_mul(
                                        pt[:, off : off + mw],
                                        pt[:, off : off + mw],
                                        u[:, :mw],
                                    )
                                vs = (h2 * NKT + kt) * VC
                                nc.tensor.matmul(
                                    o_ps[:, QB - w :],
                                    v_sb[:, vs : vs + VC],
                                    pt[:, off : off + w],
                                    start=(kt == first_kt),
                                    stop=(kt == last_kt),
                                )
                        o_sb = obp.tile([VC, QB], f32, tag="o_sb")
                        nc.vector.tensor_copy(o_sb[:], o_ps[:])
                        nc.sync.dma_start(out_ap[h, qb], o_sb[:])
    nc.compile()
    return nc


def kernel(Q, K, V, padding_mask, attention_mask):
    """Full-input entry point: shards heads across 8 cores internally.

    padding_mask is all-True and attention_mask is the causal tril for this
    module config; causality is implemented directly in the device kernel.
    """
    try:  # absent in slim containers; run_bass_kernel_spmd imports it when
        import antenv.axon_hooks  # noqa: F401  # BASS_TRACE is set
    except ImportError:
        import sys as _sys
        import types as _types

        _m = _types.ModuleType("antenv.axon_hooks")
        _m.get_axon_ntff_profile_hook = lambda: None
        _sys.modules["antenv.axon_hooks"] = _m

    from concourse.bass_utils import run_bass_kernel_spmd

    if "nc" not in _CACHE:
        _CACHE["nc"] = _build()
    nc = _CACHE["nc"]

    Qh = np.asarray(Q, dtype=np.float32).reshape(B * H, S, D)
    Kh = np.asarray(K, dtype=np.float32).reshape(B * H, S, D)
    Vh = np.asarray(V, dtype=np.float32).reshape(B * H, S, D)

    in_maps = []
    for c in range(N_CORES):
        sl = slice(c * HPC, (c + 1) * HPC)
        # [HPC, S, D] -> [HPC, D, S] -> [PAIRS, 128, S]
        qt = np.ascontiguousarray(Qh[sl].transpose(0, 2, 1)).reshape(PAIRS, 128, S)
        kt = np.ascontiguousarray(Kh[sl].transpose(0, 2, 1)).reshape(PAIRS, 128, S)
        # V + ones column: [HPC, S, VC] -> [PAIRS, 2, NKT, 128, VC]
        vv = np.concatenate(
            [Vh[sl], np.ones((HPC, S, 1), dtype=np.float32)], axis=-1
        ).reshape(PAIRS, 2, NKT, 128, VC)
        # -> [PAIRS, 128(p), 2(h2), NKT, VC]
        vv = np.ascontiguousarray(vv.transpose(0, 3, 1, 2, 4)).reshape(
            PAIRS, 128, 2 * NKT * VC
        )
        in_maps.append({"qt": qt, "kt": kt, "v": vv})

    res = run_bass_kernel_spmd(nc, in_maps, core_ids=list(range(N_CORES)))
    kernel.last_results = res

    out = np.empty((B * H, S, D), dtype=np.float32)
    for c in range(N_CORES):
        o = res.results[c]["out"]  # [HPC, NQB, VC, QB]
        num = o[:, :, :D, :]      # [HPC, NQB, D, QB]
        den = o[:, :, D:, :]      # [HPC, NQB, 1, QB]
        oc = (num / den).transpose(0, 1, 3, 2).reshape(HPC, S, D)
        out[c * HPC : (c + 1) * HPC] = oc
    return out.reshape(B, H, S, D)
